# revision 15
# baseline (speedup 1.0000x reference)
"""HalfKA NNUE forward pass on 8 Trainium2 NeuronCores — seg-matmul v15.

Network (fp32 reference):
    h1  = relu(x @ W1.T + b1)     x:[2048, 98304] sparse 0/1 (~32 nnz/row), W1:[256, 98304]
    h2  = relu(h1 @ W2.T + b2)    W2:[32, 256]
    out = h2 @ Wout.T + bout      Wout:[1, 32]  -> [2048, 1]

Strategy: data-parallel over the batch; each core handles 256 rows split into
two 128-row groups (A, B). The host packs the active-feature W1T rows of each
group row-major into a dense bf16 tensor G (contiguous DMA at full HBM rate).
G's first two tiles are metadata: tile 0 holds the per-tile segment map (the
group-local batch slot of each packed row) in cols [0, T) and an iota row in
cols [128, 256); tile 1 holds the f32 transpose identity (bitcast as bf16
pairs). A one-hot selection matrix S_t[k, b] = (seg_t[k] == b) is built in
large batched DVE is_equal ops over broadcast APs — gated only by G chunk 0's
arrival, never by a small-DMA completion. fc1 runs with S stationary:

    psum_g[b, d] += S_t[k, b].T @ G_t[k, :]     (one matmul per tile)

giving h1 batch-major; PE transposes flip it to d-major for the tiny fc2/fc3
tail. Group A's tail is emitted under group B's matmul phase; all activations
run on DVE (add+max) so no activation table load; GpSimd does no compute
(only DMA descriptor generation). Each core writes its own 256 outputs.
"""

import sys

sys.path.insert(0, "/opt/trn_rl_repo")

from contextlib import ExitStack

import numpy as np
import ml_dtypes

import concourse.bass as bass
import concourse.tile as tile
from concourse import bacc, mybir
from concourse.bass_utils import run_bass_kernel_spmd

f32 = mybir.dt.float32
bf16 = mybir.dt.bfloat16

N_CORES = 8
B = 2048
IN_DIM = 98304
H1 = 256
H2 = 32
P = 128
NMETA = 2                        # metadata tiles at the head of G

RPC = B // N_CORES               # 256 rows per core
NG = 2                           # groups per core (128 rows each)
M_T = H1 // P                    # 2 h1 partition-tiles
PAD_SEG = 256.0                  # seg value for pad slots (matches no column)

_CACHED = {}


def _chunk_sizes(total):
    """Tiles per dma_start; uniform small chunks keep the single-queue
    pipeline fine-grained so the PE never builds a backlog."""
    sizes = [4]
    rem = total - 4
    while rem > 4:
        sizes.append(4)
        rem -= 4
    if rem > 0:
        sizes.append(rem)
    return sizes


def _s_batches(t_tot):
    """S-build batch sizes: small first so matmul 0 starts early."""
    sizes = [4, 8]
    rem = t_tot - 12
    while rem > 12:
        sizes.append(12)
        rem -= 12
    if rem > 0:
        sizes.append(rem)
    return sizes


def _build_program(t_ga, t_gb, debug=False):
    t_tot = t_ga + 2 * t_gb
    # (start tile, n tiles, batch width, output column offset)
    groups = [
        (0, t_ga, P, 0),
        (t_ga, t_gb, P // 2, P),
        (t_ga + t_gb, t_gb, P // 2, P + P // 2),
    ]

    nc = bacc.Bacc(
        "TRN2",
        target_bir_lowering=False,
        debug=debug,
        num_devices=N_CORES,
    )

    g_d = nc.dram_tensor("g", [P, NMETA + t_tot, H1], bf16, kind="ExternalInput")
    b1_d = nc.dram_tensor("b1", [P, M_T], f32, kind="ExternalInput")
    w2t_d = nc.dram_tensor("w2t", [P, M_T, H2], f32, kind="ExternalInput")
    b2_d = nc.dram_tensor("b2", [H2, 1], f32, kind="ExternalInput")
    woutt_d = nc.dram_tensor("woutt", [H2 + 1, 1], f32, kind="ExternalInput")
    out_d = nc.dram_tensor("out", [1, RPC], f32, kind="ExternalOutput")

    chunks = []
    pos = 0
    for c in _chunk_sizes(NMETA + t_tot):
        chunks.append((pos, pos + c))
        pos += c
    assert pos == NMETA + t_tot, (pos, t_tot)

    with tile.TileContext(nc) as tc:
        with ExitStack() as ctx:
            const = ctx.enter_context(tc.tile_pool(name="const", bufs=1))
            gpool = ctx.enter_context(tc.tile_pool(name="gp", bufs=1))
            spool = ctx.enter_context(tc.tile_pool(name="sp", bufs=1))
            hpool = ctx.enter_context(tc.tile_pool(name="hp", bufs=2))
            apool = ctx.enter_context(tc.tile_pool(name="ap", bufs=2))
            smp = ctx.enter_context(tc.tile_pool(name="small", bufs=4))
            ps_h = ctx.enter_context(tc.tile_pool(name="psh", bufs=1, space="PSUM"))
            ps_t = ctx.enter_context(tc.tile_pool(name="pst", bufs=1, space="PSUM"))
            ps_2 = ctx.enter_context(tc.tile_pool(name="ps2", bufs=1, space="PSUM"))
            ps_3 = ctx.enter_context(tc.tile_pool(name="ps3", bufs=1, space="PSUM"))

            # all G chunks on ONE queue: strict FIFO transfers mean each
            # chunk's completion semaphore fires as soon as its own bytes are
            # done, instead of lagging behind interleaved later chunks
            gt = gpool.tile([P, NMETA + t_tot, H1], bf16, name="gt", tag="gt")
            for i, (t0, t1) in enumerate(chunks):
                nc.sync.dma_start(
                    gt[:, t0:t1, :], g_d.ap()[:, t0:t1, :]
                )

            # small weights after the first chunks on the scalar queue (all
            # are consumed late, far off the critical path)
            b1_s = const.tile([P, M_T], f32)
            nc.scalar.dma_start(b1_s[:], b1_d.ap())
            w2t_s = const.tile([P, M_T, H2], f32)
            nc.scalar.dma_start(w2t_s[:], w2t_d.ap())
            b2_s = const.tile([H2, 1], f32)
            nc.scalar.dma_start(b2_s[:], b2_d.ap())
            woutt_s = const.tile([H2 + 1, 1], f32)
            nc.scalar.dma_start(woutt_s[:], woutt_d.ap())

            # metadata views into G's head tiles
            iota_ap = gt[:, 1, P:2 * P]                  # [P, P] bf16
            ident_ap = gt[:, 1, 0:P]                     # [P, P] bf16

            # S build: batched DVE is_equal over broadcast APs,
            # S[p, t, b] = (iota[p, b] == seg[p, t])
            st = spool.tile([P, t_tot, P], bf16, name="st", tag="st")
            s0 = 0
            for n in _s_batches(t_tot):
                s1 = s0 + n
                iota_b = iota_ap.unsqueeze(1).broadcast_to([P, n, P])
                seg_b = gt[:, 0, s0:s1].unsqueeze(2).broadcast_to([P, n, P])
                nc.vector.tensor_tensor(
                    st[:, s0:s1, :], iota_b, seg_b, mybir.AluOpType.is_equal
                )
                s0 = s1
            assert s0 == t_tot

            # fc1: psum_g[b, d] = sum_t S_t.T @ G_t   (S stationary);
            # group A's tail (copy/transpose/relu/fc2 half) is emitted before
            # group B's matmuls so it hides under B's DMA-gated stream
            ps = [
                ps_h.tile([w, RPC], f32, tag=f"ps{g}", name=f"ps{g}")
                for g, (_, _, w, _) in enumerate(groups)
            ]
            h_sb = [
                hpool.tile([w, RPC], bf16, name=f"hsb{g}", tag=f"hsb{g}")
                for g, (_, _, w, _) in enumerate(groups)
            ]
            psT = [
                ps_t.tile([P, M_T, w], bf16, tag=f"pst{g}", name=f"pst{g}")
                for g, (_, _, w, _) in enumerate(groups)
            ]
            acts = [
                apool.tile([P, RPC], f32, name=f"act{m}", tag=f"act{m}")
                for m in range(M_T)
            ]
            p2 = ps_2.tile([H2, RPC], f32, name="p2", tag="p2")
            h2t = smp.tile([H2 + 1, RPC], f32, tag="h2", name="h2t")
            nc.vector.memset(h2t[H2:H2 + 1, :], 1.0)
            p3 = ps_3.tile([1, RPC], f32, name="p3", tag="p3")
            ot = smp.tile([1, RPC], f32, tag="ot", name="ot")

            def tail(g):
                # entirely on Scalar + PE so it never contends with the DVE
                # S-build queue: psum -> sbuf, transpose to d-major,
                # bias+relu, fc2 slice, fc3 slice, out slice
                _, _, w, co = groups[g]
                nc.scalar.activation(
                    h_sb[g][:], ps[g][:], mybir.ActivationFunctionType.Copy
                )
                for m in range(M_T):
                    nc.tensor.matmul(
                        psT[g][:, m, :],
                        h_sb[g][:, m * P:(m + 1) * P],
                        ident_ap if w == P else gt[0:w, 1, 0:w],
                        is_transpose=True,
                    )
                for m in range(M_T):
                    nc.scalar.activation(
                        acts[m][:, co:co + w],
                        psT[g][:, m, :],
                        mybir.ActivationFunctionType.Relu,
                        bias=b1_s[:, m:m + 1],
                    )
                for m in range(M_T):
                    nc.tensor.matmul(
                        p2[:, co:co + w],
                        w2t_s[:, m, :],
                        acts[m][:, co:co + w],
                        start=(m == 0), stop=(m == M_T - 1),
                    )
                nc.scalar.activation(
                    h2t[0:H2, co:co + w],
                    p2[:, co:co + w],
                    mybir.ActivationFunctionType.Relu,
                    bias=b2_s[:],
                )
                nc.tensor.matmul(
                    p3[:, co:co + w], woutt_s[:],
                    h2t[:, co:co + w], start=True, stop=True,
                )
                nc.scalar.activation(
                    ot[:, co:co + w], p3[:, co:co + w],
                    mybir.ActivationFunctionType.Copy,
                )
                nc.sync.dma_start(
                    out_d.ap()[0, co:co + w],
                    ot[:, co:co + w],
                )

            for g, (tstart, nt, w, co) in enumerate(groups):
                for t in range(nt):
                    ti = tstart + t
                    nc.tensor.matmul(
                        ps[g][:],
                        st[:, ti, 0:w],
                        gt[:, NMETA + ti, :],
                        start=(t == 0),
                        stop=(t == nt - 1),
                    )
                tail(g)

    nc.compile()
    return nc


def get_program(t_ga, t_gb, debug=False):
    key = ("nc", t_ga, t_gb, debug)
    if key not in _CACHED:
        _CACHED[key] = _build_program(t_ga, t_gb, debug)
    return _CACHED[key]


def _deal(items, n_bins, weights):
    """Snake-deal items into n_bins by descending weight to equalize sums."""
    order = np.argsort(-weights, kind="stable")
    bins = [[] for _ in range(n_bins)]
    for i, idx in enumerate(order):
        c = i % (2 * n_bins)
        bins[c if c < n_bins else 2 * n_bins - 1 - c].append(items[idx])
    return bins


def _prep_inputs(x, W1, b1, W2, b2, Wout, bout):
    bf = ml_dtypes.bfloat16

    w1t_h = np.ascontiguousarray(W1.T.astype(bf))                # [98304, 256]
    b1_h = np.ascontiguousarray(b1.reshape(M_T, P).T)            # [P, M_T]
    w2t_h = np.ascontiguousarray(W2.T.reshape(M_T, P, H2).transpose(1, 0, 2))
    b2_h = np.ascontiguousarray(b2.reshape(H2, 1))
    woutt_h = np.concatenate(
        [Wout.T, bout.reshape(1, 1)], axis=0
    ).astype(np.float32)                                         # [H2+1, 1]

    rows_all, cols_all = np.nonzero(x != 0.0)
    nnz = np.bincount(rows_all, minlength=B)
    bounds = np.searchsorted(rows_all, np.arange(B + 1))
    feat_of = [cols_all[bounds[r]:bounds[r + 1]] for r in range(B)]

    # deal rows into 16 (core, half) bins; each core gets one 128-row group
    # (A) and one group split into two 64-row subgroups (B1, B2) so the
    # final serial tail is half-width
    bins = _deal(np.arange(B), N_CORES * NG, nnz)
    core_groups = []            # per core: list of (rows, width)
    for c in range(N_CORES):
        arows = np.array(bins[c * NG])
        brows = np.array(bins[c * NG + 1])
        bhalves = _deal(brows, 2, nnz[brows])
        core_groups.append([
            (arows, P),
            (np.array(bhalves[0]), P // 2),
            (np.array(bhalves[1]), P // 2),
        ])
    t_ga = max(
        1,
        max(
            (int(nnz[g[0][0]].sum()) + P - 1) // P for g in core_groups
        ),
    )
    t_gb = max(
        1,
        max(
            (int(nnz[rows].sum()) + P - 1) // P
            for g in core_groups
            for rows, w in g[1:]
        ),
    )
    t_tot = t_ga + 2 * t_gb
    assert t_tot <= 2 * P, "seg map must fit in meta tile 0"

    out_pos = np.empty(B, dtype=np.int64)   # global row -> flat result index
    in_maps = []
    for c in range(N_CORES):
        data = np.zeros((t_tot * P, H1), dtype=bf)
        seg = np.full((t_tot * P,), PAD_SEG, dtype=np.float32)
        col_off = [0, P, P + P // 2]
        bases = [0, t_ga * P, (t_ga + t_gb) * P]
        for g, (grows, w) in enumerate(core_groups[c]):
            fs = np.concatenate([feat_of[r] for r in grows])
            bs = np.concatenate(
                [np.full(len(feat_of[r]), s) for s, r in enumerate(grows)]
            )
            base = bases[g]
            data[base:base + len(fs)] = w1t_h[fs]
            seg[base:base + len(fs)] = bs
            for s, r in enumerate(grows):
                out_pos[r] = c * RPC + col_off[g] + s
        # meta tile 0: seg map in cols [0, t_tot)
        meta0 = np.zeros((P, H1), dtype=bf)
        meta0[:, 0:t_tot] = seg.reshape(t_tot, P).T
        # meta tile 1: bf16 identity in cols [0, 128), iota in [128, 256)
        meta1 = np.zeros((P, H1), dtype=bf)
        meta1[:, 0:P] = np.eye(P, dtype=np.float32)
        meta1[:, P:2 * P] = np.arange(P, dtype=np.float32)[None, :]
        g_h = np.concatenate(
            [
                meta0[:, None, :],
                meta1[:, None, :],
                data.reshape(t_tot, P, H1).transpose(1, 0, 2),
            ],
            axis=1,
        )
        in_maps.append({
            "g": np.ascontiguousarray(g_h),
            "b1": b1_h,
            "w2t": w2t_h,
            "b2": b2_h,
            "woutt": woutt_h,
        })
    return in_maps, out_pos, t_ga, t_gb


def kernel(x, W1, b1, W2, b2, Wout, bout, _trace=False, _trace_kwargs=None):
    x = np.asarray(x, dtype=np.float32)
    W1 = np.asarray(W1, dtype=np.float32)
    b1 = np.asarray(b1, dtype=np.float32)
    W2 = np.asarray(W2, dtype=np.float32)
    b2 = np.asarray(b2, dtype=np.float32)
    Wout = np.asarray(Wout, dtype=np.float32)
    bout = np.asarray(bout, dtype=np.float32)

    in_maps, out_pos, t_ga, t_gb = _prep_inputs(x, W1, b1, W2, b2, Wout, bout)
    nc = get_program(t_ga, t_gb)
    res = run_bass_kernel_spmd(
        nc,
        in_maps,
        core_ids=list(range(N_CORES)),
        trace=_trace,
        **(_trace_kwargs or {}),
    )
    flat = np.concatenate(
        [res.results[c]["out"].reshape(RPC) for c in range(N_CORES)]
    )
    out = flat[out_pos].reshape(B, 1).astype(np.float32)
    if _trace:
        kernel.last_results = res
    return out


if __name__ == "__main__":
    rng = np.random.default_rng(0)
    x = (rng.random((B, IN_DIM)) < 32.0 / IN_DIM).astype(np.float32)
    W1 = rng.standard_normal((H1, IN_DIM), dtype=np.float32) / np.sqrt(IN_DIM)
    b1 = rng.standard_normal(H1, dtype=np.float32) / np.sqrt(IN_DIM)
    W2 = rng.standard_normal((H2, H1), dtype=np.float32) / np.sqrt(H1)
    b2 = rng.standard_normal(H2, dtype=np.float32) / np.sqrt(H1)
    Wout = rng.standard_normal((1, H2), dtype=np.float32) / np.sqrt(H2)
    bout = rng.standard_normal(1, dtype=np.float32) / np.sqrt(H2)
    got = kernel(x, W1, b1, W2, b2, Wout, bout)
    h1 = np.maximum(x @ W1.T + b1, 0)
    h2 = np.maximum(h1 @ W2.T + b2, 0)
    exp = h2 @ Wout.T + bout
    print("rel err:", np.abs(got - exp).max() / np.abs(exp).max())


# revision 16
# speedup vs baseline: 1.0125x; 1.0125x over previous
"""HalfKA NNUE forward pass on 8 Trainium2 NeuronCores — seg-matmul v13.

Network (fp32 reference):
    h1  = relu(x @ W1.T + b1)     x:[2048, 98304] sparse 0/1 (~32 nnz/row), W1:[256, 98304]
    h2  = relu(h1 @ W2.T + b2)    W2:[32, 256]
    out = h2 @ Wout.T + bout      Wout:[1, 32]  -> [2048, 1]

Strategy: data-parallel over the batch; each core handles 256 rows split into
two 128-row groups (A, B). The host packs the active-feature W1T rows of each
group row-major into a dense bf16 tensor G (contiguous DMA at full HBM rate).
G's first two tiles are metadata: tile 0 holds the per-tile segment map (the
group-local batch slot of each packed row) in cols [0, T) and an iota row in
cols [128, 256); tile 1 holds the f32 transpose identity (bitcast as bf16
pairs). A one-hot selection matrix S_t[k, b] = (seg_t[k] == b) is built in
large batched DVE is_equal ops over broadcast APs — gated only by G chunk 0's
arrival, never by a small-DMA completion. fc1 runs with S stationary:

    psum_g[b, d] += S_t[k, b].T @ G_t[k, :]     (one matmul per tile)

giving h1 batch-major; PE transposes flip it to d-major for the tiny fc2/fc3
tail. Group A's tail is emitted under group B's matmul phase; all activations
run on DVE (add+max) so no activation table load; GpSimd does no compute
(only DMA descriptor generation). Each core writes its own 256 outputs.
"""

import sys

sys.path.insert(0, "/opt/trn_rl_repo")

from contextlib import ExitStack

import numpy as np
import ml_dtypes

import concourse.bass as bass
import concourse.tile as tile
from concourse import bacc, mybir
from concourse.bass_utils import run_bass_kernel_spmd

f32 = mybir.dt.float32
bf16 = mybir.dt.bfloat16

N_CORES = 8
B = 2048
IN_DIM = 98304
H1 = 256
H2 = 32
P = 128
NMETA = 2                        # metadata tiles at the head of G

RPC = B // N_CORES               # 256 rows per core
NG = 2                           # groups per core (128 rows each)
M_T = H1 // P                    # 2 h1 partition-tiles
PAD_SEG = 256.0                  # seg value for pad slots (matches no column)

_CACHED = {}


def _chunk_sizes(total):
    """Tiles per dma_start; uniform small chunks keep the single-queue
    pipeline fine-grained so the PE never builds a backlog."""
    sizes = [4]
    rem = total - 4
    while rem > 4:
        sizes.append(4)
        rem -= 4
    if rem > 0:
        sizes.append(rem)
    return sizes


def _s_batches(t_tot):
    """S-build batch sizes: small first so matmul 0 starts early."""
    sizes = [4, 8]
    rem = t_tot - 12
    while rem > 12:
        sizes.append(12)
        rem -= 12
    if rem > 0:
        sizes.append(rem)
    return sizes


def _build_program(t_g, debug=False):
    t_tot = NG * t_g

    nc = bacc.Bacc(
        "TRN2",
        target_bir_lowering=False,
        debug=debug,
        num_devices=N_CORES,
    )

    g_d = nc.dram_tensor("g", [P, NMETA + t_tot, H1], bf16, kind="ExternalInput")
    b1_d = nc.dram_tensor("b1", [P, M_T], f32, kind="ExternalInput")
    w2t_d = nc.dram_tensor("w2t", [P, M_T, H2], f32, kind="ExternalInput")
    b2_d = nc.dram_tensor("b2", [H2, 1], f32, kind="ExternalInput")
    woutt_d = nc.dram_tensor("woutt", [H2 + 1, 1], f32, kind="ExternalInput")
    out_d = nc.dram_tensor("out", [1, RPC], f32, kind="ExternalOutput")

    chunks = []
    pos = 0
    for c in _chunk_sizes(NMETA + t_tot):
        chunks.append((pos, pos + c))
        pos += c
    assert pos == NMETA + t_tot, (pos, t_tot)

    with tile.TileContext(nc) as tc:
        with ExitStack() as ctx:
            const = ctx.enter_context(tc.tile_pool(name="const", bufs=1))
            gpool = ctx.enter_context(tc.tile_pool(name="gp", bufs=1))
            spool = ctx.enter_context(tc.tile_pool(name="sp", bufs=1))
            hpool = ctx.enter_context(tc.tile_pool(name="hp", bufs=2))
            apool = ctx.enter_context(tc.tile_pool(name="ap", bufs=2))
            smp = ctx.enter_context(tc.tile_pool(name="small", bufs=4))
            ps_h = ctx.enter_context(tc.tile_pool(name="psh", bufs=1, space="PSUM"))
            ps_t = ctx.enter_context(tc.tile_pool(name="pst", bufs=1, space="PSUM"))
            ps_2 = ctx.enter_context(tc.tile_pool(name="ps2", bufs=1, space="PSUM"))
            ps_3 = ctx.enter_context(tc.tile_pool(name="ps3", bufs=1, space="PSUM"))

            # all G chunks on ONE queue: strict FIFO transfers mean each
            # chunk's completion semaphore fires as soon as its own bytes are
            # done, instead of lagging behind interleaved later chunks
            gt = gpool.tile([P, NMETA + t_tot, H1], bf16, name="gt", tag="gt")
            for i, (t0, t1) in enumerate(chunks):
                nc.sync.dma_start(
                    gt[:, t0:t1, :], g_d.ap()[:, t0:t1, :]
                )

            # small weights after the first chunks on the scalar queue (all
            # are consumed late, far off the critical path)
            b1_s = const.tile([P, M_T], f32)
            nc.scalar.dma_start(b1_s[:], b1_d.ap())
            w2t_s = const.tile([P, M_T, H2], f32)
            nc.scalar.dma_start(w2t_s[:], w2t_d.ap())
            b2_s = const.tile([H2, 1], f32)
            nc.scalar.dma_start(b2_s[:], b2_d.ap())
            woutt_s = const.tile([H2 + 1, 1], f32)
            nc.scalar.dma_start(woutt_s[:], woutt_d.ap())

            # metadata views into G's head tiles
            iota_ap = gt[:, 1, P:2 * P]                  # [P, P] bf16
            ident_ap = gt[:, 1, 0:P]                     # [P, P] bf16

            # S build: batched DVE is_equal over broadcast APs,
            # S[p, t, b] = (iota[p, b] == seg[p, t])
            st = spool.tile([P, t_tot, P], bf16, name="st", tag="st")
            s0 = 0
            for n in _s_batches(t_tot):
                s1 = s0 + n
                iota_b = iota_ap.unsqueeze(1).broadcast_to([P, n, P])
                seg_b = gt[:, 0, s0:s1].unsqueeze(2).broadcast_to([P, n, P])
                nc.vector.tensor_tensor(
                    st[:, s0:s1, :], iota_b, seg_b, mybir.AluOpType.is_equal
                )
                s0 = s1
            assert s0 == t_tot

            # fc1: psum_g[b, d] = sum_t S_t.T @ G_t   (S stationary);
            # group A's tail (copy/transpose/relu/fc2 half) is emitted before
            # group B's matmuls so it hides under B's DMA-gated stream
            ps = [
                ps_h.tile([P, RPC], f32, tag=f"ps{g}", name=f"ps{g}")
                for g in range(NG)
            ]
            h_sb = [
                hpool.tile([P, RPC], bf16, name=f"hsb{g}", tag=f"hsb{g}")
                for g in range(NG)
            ]
            psT = [
                [
                    ps_t.tile([P, P], bf16, tag=f"pst{m}{g}", name=f"pst{m}{g}")
                    for g in range(NG)
                ]
                for m in range(M_T)
            ]
            acts = [
                apool.tile([P, RPC], f32, name=f"act{m}", tag=f"act{m}")
                for m in range(M_T)
            ]
            p2 = ps_2.tile([H2, RPC], f32, name="p2", tag="p2")
            h2t = smp.tile([H2 + 1, RPC], f32, tag="h2", name="h2t")
            nc.vector.memset(h2t[H2:H2 + 1, :], 1.0)
            p3 = ps_3.tile([1, RPC], f32, name="p3", tag="p3")
            ot = smp.tile([1, RPC], f32, tag="ot", name="ot")

            def tail(g):
                # entirely on Scalar + PE so it never contends with the DVE
                # S-build queue: psum -> sbuf, transpose to d-major,
                # bias+relu, fc2 half, fc3 half, out half
                nc.scalar.activation(
                    h_sb[g][:], ps[g][:], mybir.ActivationFunctionType.Copy
                )
                for m in range(M_T):
                    nc.tensor.transpose(
                        psT[m][g][:],
                        h_sb[g][:, m * P:(m + 1) * P],
                        ident_ap,
                    )
                for m in range(M_T):
                    nc.scalar.activation(
                        acts[m][:, g * P:(g + 1) * P],
                        psT[m][g][:],
                        mybir.ActivationFunctionType.Relu,
                        bias=b1_s[:, m:m + 1],
                    )
                for m in range(M_T):
                    nc.tensor.matmul(
                        p2[:, g * P:(g + 1) * P],
                        w2t_s[:, m, :],
                        acts[m][:, g * P:(g + 1) * P],
                        start=(m == 0), stop=(m == M_T - 1),
                    )
                nc.scalar.activation(
                    h2t[0:H2, g * P:(g + 1) * P],
                    p2[:, g * P:(g + 1) * P],
                    mybir.ActivationFunctionType.Relu,
                    bias=b2_s[:],
                )
                nc.tensor.matmul(
                    p3[:, g * P:(g + 1) * P], woutt_s[:],
                    h2t[:, g * P:(g + 1) * P], start=True, stop=True,
                )
                nc.scalar.activation(
                    ot[:, g * P:(g + 1) * P], p3[:, g * P:(g + 1) * P],
                    mybir.ActivationFunctionType.Copy,
                )
                nc.sync.dma_start(
                    out_d.ap()[0, g * P:(g + 1) * P],
                    ot[:, g * P:(g + 1) * P],
                )

            for g in range(NG):
                for t in range(t_g):
                    ti = g * t_g + t
                    nc.tensor.matmul(
                        ps[g][:],
                        st[:, ti, :],
                        gt[:, NMETA + ti, :],
                        start=(t == 0),
                        stop=(t == t_g - 1),
                    )
                tail(g)

    nc.compile()
    return nc


def get_program(t_g, debug=False):
    key = ("nc", t_g, debug)
    if key not in _CACHED:
        _CACHED[key] = _build_program(t_g, debug)
    return _CACHED[key]


def _deal(items, n_bins, weights):
    """Snake-deal items into n_bins by descending weight to equalize sums."""
    order = np.argsort(-weights, kind="stable")
    bins = [[] for _ in range(n_bins)]
    for i, idx in enumerate(order):
        c = i % (2 * n_bins)
        bins[c if c < n_bins else 2 * n_bins - 1 - c].append(items[idx])
    return bins


def _prep_inputs(x, W1, b1, W2, b2, Wout, bout):
    bf = ml_dtypes.bfloat16

    w1t_h = np.ascontiguousarray(W1.T.astype(bf))                # [98304, 256]
    b1_h = np.ascontiguousarray(b1.reshape(M_T, P).T)            # [P, M_T]
    w2t_h = np.ascontiguousarray(W2.T.reshape(M_T, P, H2).transpose(1, 0, 2))
    b2_h = np.ascontiguousarray(b2.reshape(H2, 1))
    woutt_h = np.concatenate(
        [Wout.T, bout.reshape(1, 1)], axis=0
    ).astype(np.float32)                                         # [H2+1, 1]

    rows_all, cols_all = np.nonzero(x != 0.0)
    nnz = np.bincount(rows_all, minlength=B)
    bounds = np.searchsorted(rows_all, np.arange(B + 1))
    feat_of = [cols_all[bounds[r]:bounds[r + 1]] for r in range(B)]

    # deal rows into the 16 (core, group) bins directly to minimize the max
    # bin size (which sets the tile count and thus the DMA volume)
    bins = _deal(np.arange(B), N_CORES * NG, nnz)
    max_cnt = max(int(nnz[bs].sum()) for bs in bins)
    t_g = max(1, (max_cnt + P - 1) // P)
    t_tot = NG * t_g
    assert t_tot <= 2 * P, "seg map must fit in meta tile 0"
    cap = t_g * P

    out_pos = np.empty(B, dtype=np.int64)   # global row -> flat result index
    in_maps = []
    for c in range(N_CORES):
        data = np.zeros((t_tot * P, H1), dtype=bf)
        seg = np.full((t_tot * P,), PAD_SEG, dtype=np.float32)
        for g in range(NG):
            grows = bins[c * NG + g]
            assert len(grows) == P, len(grows)
            fs = np.concatenate([feat_of[r] for r in grows])
            bs = np.concatenate(
                [np.full(len(feat_of[r]), s) for s, r in enumerate(grows)]
            )
            assert len(fs) <= cap
            base = g * cap
            data[base:base + len(fs)] = w1t_h[fs]
            seg[base:base + len(fs)] = bs
            for s, r in enumerate(grows):
                out_pos[r] = c * RPC + g * P + s
        # meta tile 0: seg map in cols [0, t_tot)
        meta0 = np.zeros((P, H1), dtype=bf)
        meta0[:, 0:t_tot] = seg.reshape(t_tot, P).T
        # meta tile 1: bf16 identity in cols [0, 128), iota in [128, 256)
        meta1 = np.zeros((P, H1), dtype=bf)
        meta1[:, 0:P] = np.eye(P, dtype=np.float32)
        meta1[:, P:2 * P] = np.arange(P, dtype=np.float32)[None, :]
        g_h = np.concatenate(
            [
                meta0[:, None, :],
                meta1[:, None, :],
                data.reshape(t_tot, P, H1).transpose(1, 0, 2),
            ],
            axis=1,
        )
        in_maps.append({
            "g": np.ascontiguousarray(g_h),
            "b1": b1_h,
            "w2t": w2t_h,
            "b2": b2_h,
            "woutt": woutt_h,
        })
    return in_maps, out_pos, t_g


def kernel(x, W1, b1, W2, b2, Wout, bout, _trace=False, _trace_kwargs=None):
    x = np.asarray(x, dtype=np.float32)
    W1 = np.asarray(W1, dtype=np.float32)
    b1 = np.asarray(b1, dtype=np.float32)
    W2 = np.asarray(W2, dtype=np.float32)
    b2 = np.asarray(b2, dtype=np.float32)
    Wout = np.asarray(Wout, dtype=np.float32)
    bout = np.asarray(bout, dtype=np.float32)

    in_maps, out_pos, t_g = _prep_inputs(x, W1, b1, W2, b2, Wout, bout)
    nc = get_program(t_g)
    res = run_bass_kernel_spmd(
        nc,
        in_maps,
        core_ids=list(range(N_CORES)),
        trace=_trace,
        **(_trace_kwargs or {}),
    )
    flat = np.concatenate(
        [res.results[c]["out"].reshape(RPC) for c in range(N_CORES)]
    )
    out = flat[out_pos].reshape(B, 1).astype(np.float32)
    if _trace:
        kernel.last_results = res
    return out


if __name__ == "__main__":
    rng = np.random.default_rng(0)
    x = (rng.random((B, IN_DIM)) < 32.0 / IN_DIM).astype(np.float32)
    W1 = rng.standard_normal((H1, IN_DIM), dtype=np.float32) / np.sqrt(IN_DIM)
    b1 = rng.standard_normal(H1, dtype=np.float32) / np.sqrt(IN_DIM)
    W2 = rng.standard_normal((H2, H1), dtype=np.float32) / np.sqrt(H1)
    b2 = rng.standard_normal(H2, dtype=np.float32) / np.sqrt(H1)
    Wout = rng.standard_normal((1, H2), dtype=np.float32) / np.sqrt(H2)
    bout = rng.standard_normal(1, dtype=np.float32) / np.sqrt(H2)
    got = kernel(x, W1, b1, W2, b2, Wout, bout)
    h1 = np.maximum(x @ W1.T + b1, 0)
    h2 = np.maximum(h1 @ W2.T + b2, 0)
    exp = h2 @ Wout.T + bout
    print("rel err:", np.abs(got - exp).max() / np.abs(exp).max())


# revision 17
# speedup vs baseline: 1.0881x; 1.0747x over previous
"""HalfKA NNUE forward pass on 8 Trainium2 NeuronCores — seg-matmul v16.

Network (fp32 reference):
    h1  = relu(x @ W1.T + b1)     x:[2048, 98304] sparse 0/1 (~32 nnz/row), W1:[256, 98304]
    h2  = relu(h1 @ W2.T + b2)    W2:[32, 256]
    out = h2 @ Wout.T + bout      Wout:[1, 32]  -> [2048, 1]

Strategy: data-parallel over the batch; each core handles 256 rows split into
two 128-row groups (A, B). The host packs the active-feature W1T rows of each
group row-major into a dense bf16 tensor G (contiguous DMA at full HBM rate).
G's first two tiles are metadata: tile 0 holds the per-tile segment map (the
group-local batch slot of each packed row) in cols [0, T) and an iota row in
cols [128, 256); tile 1 holds the f32 transpose identity (bitcast as bf16
pairs). A one-hot selection matrix S_t[k, b] = (seg_t[k] == b) is built in
large batched DVE is_equal ops over broadcast APs — gated only by G chunk 0's
arrival, never by a small-DMA completion. fc1 runs with S stationary:

    psum_g[b, d] += S_t[k, b].T @ G_t[k, :]     (one matmul per tile)

giving h1 batch-major; PE transposes flip it to d-major for the tiny fc2/fc3
tail. Group A's tail is emitted under group B's matmul phase; all activations
run on DVE (add+max) so no activation table load; GpSimd does no compute
(only DMA descriptor generation). Each core writes its own 256 outputs.
"""

import sys

sys.path.insert(0, "/opt/trn_rl_repo")

from contextlib import ExitStack

import numpy as np
import ml_dtypes

import concourse.bass as bass
import concourse.tile as tile
from concourse import bacc, mybir
from concourse.bass_utils import run_bass_kernel_spmd

f32 = mybir.dt.float32
bf16 = mybir.dt.bfloat16

N_CORES = 8
B = 2048
IN_DIM = 98304
H1 = 256
H2 = 32
P = 128
NMETA = 2                        # metadata tiles at the head of G

RPC = B // N_CORES               # 256 rows per core
NG = 2                           # groups per core (128 rows each)
M_T = H1 // P                    # 2 h1 partition-tiles
PAD_SEG = 256.0                  # seg value for pad slots (matches no column)

_CACHED = {}


def _chunk_sizes(total):
    """Tiles per dma_start; uniform small chunks keep the single-queue
    pipeline fine-grained so the PE never builds a backlog."""
    sizes = [4]
    rem = total - 4
    while rem > 4:
        sizes.append(4)
        rem -= 4
    if rem > 0:
        sizes.append(rem)
    return sizes


def _s_batches(t_tot):
    """S-build batch sizes: small first so matmul 0 starts early."""
    sizes = [4, 8]
    rem = t_tot - 12
    while rem > 12:
        sizes.append(12)
        rem -= 12
    if rem > 0:
        sizes.append(rem)
    return sizes


def _build_program(t_g, debug=False):
    t_tot = NG * t_g

    nc = bacc.Bacc(
        "TRN2",
        target_bir_lowering=False,
        debug=debug,
        num_devices=N_CORES,
    )

    g_d = nc.dram_tensor("g", [P, NMETA + t_tot, H1], bf16, kind="ExternalInput")
    b1_d = nc.dram_tensor("b1", [P, M_T], f32, kind="ExternalInput")
    w2t_d = nc.dram_tensor("w2t", [P, M_T, H2], f32, kind="ExternalInput")
    b2_d = nc.dram_tensor("b2", [H2, 1], f32, kind="ExternalInput")
    woutt_d = nc.dram_tensor("woutt", [H2 + 1, 1], f32, kind="ExternalInput")
    out_d = nc.dram_tensor("out", [1, RPC], f32, kind="ExternalOutput")

    chunks = []
    pos = 0
    for c in _chunk_sizes(NMETA + t_tot):
        chunks.append((pos, pos + c))
        pos += c
    assert pos == NMETA + t_tot, (pos, t_tot)

    with tile.TileContext(nc) as tc:
        with ExitStack() as ctx:
            const = ctx.enter_context(tc.tile_pool(name="const", bufs=1))
            gpool = ctx.enter_context(tc.tile_pool(name="gp", bufs=1))
            spool = ctx.enter_context(tc.tile_pool(name="sp", bufs=1))
            hpool = ctx.enter_context(tc.tile_pool(name="hp", bufs=2))
            apool = ctx.enter_context(tc.tile_pool(name="ap", bufs=2))
            smp = ctx.enter_context(tc.tile_pool(name="small", bufs=4))
            ps_h = ctx.enter_context(tc.tile_pool(name="psh", bufs=1, space="PSUM"))
            ps_t = ctx.enter_context(tc.tile_pool(name="pst", bufs=1, space="PSUM"))
            ps_2 = ctx.enter_context(tc.tile_pool(name="ps2", bufs=1, space="PSUM"))
            ps_3 = ctx.enter_context(tc.tile_pool(name="ps3", bufs=1, space="PSUM"))

            # all G chunks on ONE queue: strict FIFO transfers mean each
            # chunk's completion semaphore fires as soon as its own bytes are
            # done, instead of lagging behind interleaved later chunks
            gt = gpool.tile([P, NMETA + t_tot, H1], bf16, name="gt", tag="gt")
            for i, (t0, t1) in enumerate(chunks):
                nc.sync.dma_start(
                    gt[:, t0:t1, :], g_d.ap()[:, t0:t1, :]
                )

            # small weights ride the SAME sync queue as G, fake-delayed so
            # the scheduler slots them mid-stream: no second queue interleaves
            # tiny descriptors into G's sequential HBM walk, and they still
            # arrive long before the tails need them
            with tc.tile_wait_until(0.012):
                b1_s = const.tile([P, M_T], f32)
                nc.sync.dma_start(b1_s[:], b1_d.ap())
                w2t_s = const.tile([P, M_T, H2], f32)
                nc.sync.dma_start(w2t_s[:], w2t_d.ap())
                b2_s = const.tile([H2, 1], f32)
                nc.sync.dma_start(b2_s[:], b2_d.ap())
                woutt_s = const.tile([H2 + 1, 1], f32)
                nc.sync.dma_start(woutt_s[:], woutt_d.ap())

            # metadata views into G's head tiles
            iota_ap = gt[:, 1, P:2 * P]                  # [P, P] bf16
            ident_ap = gt[:, 1, 0:P]                     # [P, P] bf16

            # S build: batched DVE is_equal over broadcast APs,
            # S[p, t, b] = (iota[p, b] == seg[p, t])
            st = spool.tile([P, t_tot, P], bf16, name="st", tag="st")
            s0 = 0
            for n in _s_batches(t_tot):
                s1 = s0 + n
                iota_b = iota_ap.unsqueeze(1).broadcast_to([P, n, P])
                seg_b = gt[:, 0, s0:s1].unsqueeze(2).broadcast_to([P, n, P])
                nc.vector.tensor_tensor(
                    st[:, s0:s1, :], iota_b, seg_b, mybir.AluOpType.is_equal
                )
                s0 = s1
            assert s0 == t_tot

            # fc1: psum_g[b, d] = sum_t S_t.T @ G_t   (S stationary);
            # group A's tail (copy/transpose/relu/fc2 half) is emitted before
            # group B's matmuls so it hides under B's DMA-gated stream
            ps = [
                ps_h.tile([P, RPC], f32, tag=f"ps{g}", name=f"ps{g}")
                for g in range(NG)
            ]
            h_sb = [
                hpool.tile([P, RPC], bf16, name=f"hsb{g}", tag=f"hsb{g}")
                for g in range(NG)
            ]
            psT = [
                [
                    ps_t.tile([P, P], bf16, tag=f"pst{m}{g}", name=f"pst{m}{g}")
                    for g in range(NG)
                ]
                for m in range(M_T)
            ]
            acts = [
                apool.tile([P, RPC], f32, name=f"act{m}", tag=f"act{m}")
                for m in range(M_T)
            ]
            p2 = ps_2.tile([H2, RPC], f32, name="p2", tag="p2")
            h2t = smp.tile([H2 + 1, RPC], f32, tag="h2", name="h2t")
            nc.vector.memset(h2t[H2:H2 + 1, :], 1.0)
            p3 = ps_3.tile([1, RPC], f32, name="p3", tag="p3")
            ot = smp.tile([1, RPC], f32, tag="ot", name="ot")

            def tail(g):
                # entirely on Scalar + PE so it never contends with the DVE
                # S-build queue: psum -> sbuf, transpose to d-major,
                # bias+relu, fc2 half, fc3 half, out half
                nc.scalar.activation(
                    h_sb[g][:], ps[g][:], mybir.ActivationFunctionType.Copy
                )
                for m in range(M_T):
                    nc.tensor.transpose(
                        psT[m][g][:],
                        h_sb[g][:, m * P:(m + 1) * P],
                        ident_ap,
                    )
                for m in range(M_T):
                    nc.scalar.activation(
                        acts[m][:, g * P:(g + 1) * P],
                        psT[m][g][:],
                        mybir.ActivationFunctionType.Relu,
                        bias=b1_s[:, m:m + 1],
                    )
                for m in range(M_T):
                    nc.tensor.matmul(
                        p2[:, g * P:(g + 1) * P],
                        w2t_s[:, m, :],
                        acts[m][:, g * P:(g + 1) * P],
                        start=(m == 0), stop=(m == M_T - 1),
                    )
                nc.scalar.activation(
                    h2t[0:H2, g * P:(g + 1) * P],
                    p2[:, g * P:(g + 1) * P],
                    mybir.ActivationFunctionType.Relu,
                    bias=b2_s[:],
                )
                nc.tensor.matmul(
                    p3[:, g * P:(g + 1) * P], woutt_s[:],
                    h2t[:, g * P:(g + 1) * P], start=True, stop=True,
                )
                nc.scalar.activation(
                    ot[:, g * P:(g + 1) * P], p3[:, g * P:(g + 1) * P],
                    mybir.ActivationFunctionType.Copy,
                )
                nc.sync.dma_start(
                    out_d.ap()[0, g * P:(g + 1) * P],
                    ot[:, g * P:(g + 1) * P],
                )

            for g in range(NG):
                for t in range(t_g):
                    ti = g * t_g + t
                    nc.tensor.matmul(
                        ps[g][:],
                        st[:, ti, :],
                        gt[:, NMETA + ti, :],
                        start=(t == 0),
                        stop=(t == t_g - 1),
                    )
                tail(g)

    nc.compile()
    return nc


def get_program(t_g, debug=False):
    key = ("nc", t_g, debug)
    if key not in _CACHED:
        _CACHED[key] = _build_program(t_g, debug)
    return _CACHED[key]


def _deal(items, n_bins, weights):
    """Snake-deal items into n_bins by descending weight to equalize sums."""
    order = np.argsort(-weights, kind="stable")
    bins = [[] for _ in range(n_bins)]
    for i, idx in enumerate(order):
        c = i % (2 * n_bins)
        bins[c if c < n_bins else 2 * n_bins - 1 - c].append(items[idx])
    return bins


def _prep_inputs(x, W1, b1, W2, b2, Wout, bout):
    bf = ml_dtypes.bfloat16

    w1t_h = np.ascontiguousarray(W1.T.astype(bf))                # [98304, 256]
    b1_h = np.ascontiguousarray(b1.reshape(M_T, P).T)            # [P, M_T]
    w2t_h = np.ascontiguousarray(W2.T.reshape(M_T, P, H2).transpose(1, 0, 2))
    b2_h = np.ascontiguousarray(b2.reshape(H2, 1))
    woutt_h = np.concatenate(
        [Wout.T, bout.reshape(1, 1)], axis=0
    ).astype(np.float32)                                         # [H2+1, 1]

    rows_all, cols_all = np.nonzero(x != 0.0)
    nnz = np.bincount(rows_all, minlength=B)
    bounds = np.searchsorted(rows_all, np.arange(B + 1))
    feat_of = [cols_all[bounds[r]:bounds[r + 1]] for r in range(B)]

    # deal rows into the 16 (core, group) bins directly to minimize the max
    # bin size (which sets the tile count and thus the DMA volume)
    bins = _deal(np.arange(B), N_CORES * NG, nnz)
    max_cnt = max(int(nnz[bs].sum()) for bs in bins)
    t_g = max(1, (max_cnt + P - 1) // P)
    t_tot = NG * t_g
    assert t_tot <= 2 * P, "seg map must fit in meta tile 0"
    cap = t_g * P

    out_pos = np.empty(B, dtype=np.int64)   # global row -> flat result index
    in_maps = []
    for c in range(N_CORES):
        data = np.zeros((t_tot * P, H1), dtype=bf)
        seg = np.full((t_tot * P,), PAD_SEG, dtype=np.float32)
        for g in range(NG):
            grows = bins[c * NG + g]
            assert len(grows) == P, len(grows)
            fs = np.concatenate([feat_of[r] for r in grows])
            bs = np.concatenate(
                [np.full(len(feat_of[r]), s) for s, r in enumerate(grows)]
            )
            assert len(fs) <= cap
            base = g * cap
            data[base:base + len(fs)] = w1t_h[fs]
            seg[base:base + len(fs)] = bs
            for s, r in enumerate(grows):
                out_pos[r] = c * RPC + g * P + s
        # meta tile 0: seg map in cols [0, t_tot)
        meta0 = np.zeros((P, H1), dtype=bf)
        meta0[:, 0:t_tot] = seg.reshape(t_tot, P).T
        # meta tile 1: bf16 identity in cols [0, 128), iota in [128, 256)
        meta1 = np.zeros((P, H1), dtype=bf)
        meta1[:, 0:P] = np.eye(P, dtype=np.float32)
        meta1[:, P:2 * P] = np.arange(P, dtype=np.float32)[None, :]
        g_h = np.concatenate(
            [
                meta0[:, None, :],
                meta1[:, None, :],
                data.reshape(t_tot, P, H1).transpose(1, 0, 2),
            ],
            axis=1,
        )
        in_maps.append({
            "g": np.ascontiguousarray(g_h),
            "b1": b1_h,
            "w2t": w2t_h,
            "b2": b2_h,
            "woutt": woutt_h,
        })
    return in_maps, out_pos, t_g


def kernel(x, W1, b1, W2, b2, Wout, bout, _trace=False, _trace_kwargs=None):
    x = np.asarray(x, dtype=np.float32)
    W1 = np.asarray(W1, dtype=np.float32)
    b1 = np.asarray(b1, dtype=np.float32)
    W2 = np.asarray(W2, dtype=np.float32)
    b2 = np.asarray(b2, dtype=np.float32)
    Wout = np.asarray(Wout, dtype=np.float32)
    bout = np.asarray(bout, dtype=np.float32)

    in_maps, out_pos, t_g = _prep_inputs(x, W1, b1, W2, b2, Wout, bout)
    nc = get_program(t_g)
    res = run_bass_kernel_spmd(
        nc,
        in_maps,
        core_ids=list(range(N_CORES)),
        trace=_trace,
        **(_trace_kwargs or {}),
    )
    flat = np.concatenate(
        [res.results[c]["out"].reshape(RPC) for c in range(N_CORES)]
    )
    out = flat[out_pos].reshape(B, 1).astype(np.float32)
    if _trace:
        kernel.last_results = res
    return out


if __name__ == "__main__":
    rng = np.random.default_rng(0)
    x = (rng.random((B, IN_DIM)) < 32.0 / IN_DIM).astype(np.float32)
    W1 = rng.standard_normal((H1, IN_DIM), dtype=np.float32) / np.sqrt(IN_DIM)
    b1 = rng.standard_normal(H1, dtype=np.float32) / np.sqrt(IN_DIM)
    W2 = rng.standard_normal((H2, H1), dtype=np.float32) / np.sqrt(H1)
    b2 = rng.standard_normal(H2, dtype=np.float32) / np.sqrt(H1)
    Wout = rng.standard_normal((1, H2), dtype=np.float32) / np.sqrt(H2)
    bout = rng.standard_normal(1, dtype=np.float32) / np.sqrt(H2)
    got = kernel(x, W1, b1, W2, b2, Wout, bout)
    h1 = np.maximum(x @ W1.T + b1, 0)
    h2 = np.maximum(h1 @ W2.T + b2, 0)
    exp = h2 @ Wout.T + bout
    print("rel err:", np.abs(got - exp).max() / np.abs(exp).max())
